# revision 1
# baseline (speedup 1.0000x reference)
"""BatchAllTripletLoss on 8 Trainium2 NeuronCores.

Strategy (data-parallel over anchors x negative-columns):
  - Host: sort the batch by label (loss is permutation invariant). After the
    sort every anchor's positives live in a contiguous run of columns within
    +/-(W-1) of its own column (W = max class size).
  - 8 cores = 4 anchor blocks (128 anchors) x 2 negative halves (256 cols).
    Each core computes distances for its block via PE matmul (fp16 inputs,
    fp32 accumulate), masks positives/negatives from labels, extracts the
    per-anchor positive band with a diagonal-stride DMA, and then runs the
    hot loop: for each band position j, one fused instruction computes
    sum_n relu(pos_j - d_neg) (and one counts pos_j - d_neg > eps),
    accumulated per-anchor via the instruction's accum_out. Work is split
    across the Vector and Scalar engines.
  - Host combines 8x[128,4] partials into the two output scalars.
"""
import sys
sys.path.insert(0, "/opt/trn_rl_repo")

import numpy as np
from contextlib import ExitStack

import bass_rust
import concourse.bass as bass
import concourse.tile as tile
from concourse import bacc, mybir
from concourse.bass_utils import run_bass_kernel_spmd

F32 = mybir.dt.float32
F16 = mybir.dt.float16
Alu = mybir.AluOpType
Act = mybir.ActivationFunctionType
AX = mybir.AxisListType

B = 512          # batch
P = 128          # anchors per block (partition dim)
NH = 256         # negative columns per core (half of B)
LARGE = 1.0e6
EPS_TL = 1.0e-5  # reference's tl > 1e-5 threshold
N_CORES = 8

_cache = {}


def _build(W: int, loop_iters: int | None = None):
    """Build + compile the per-core Bass program for max class size W.

    loop_iters: if set, wrap the whole body in a hardware For_i loop
    (benchmarking only - out is rewritten every iteration)."""
    WB = 2 * W - 1            # positive band width per anchor
    WWIN = P + 2 * (W - 1)    # window columns covering all positives of a block
    # split the band between DVE (scalar_tensor_tensor) and ACT (activation)
    n_act = max(1, int(round(WB * 327.0 / (327.0 + 585.0))))
    n_dve = WB - n_act

    nc = bacc.Bacc("TRN2", target_bir_lowering=False, debug=False,
                   num_devices=N_CORES)

    lhsT_d = nc.dram_tensor("lhsT", [P, (B // P) * P], F16, kind="ExternalInput")
    rhsn_d = nc.dram_tensor("rhsn", [P, (B // P) * NH], F16, kind="ExternalInput")
    rhsw_d = nc.dram_tensor("rhsw", [P, (B // P) * WWIN], F16, kind="ExternalInput")
    laba_d = nc.dram_tensor("laba", [P, 1], F32, kind="ExternalInput")
    labn_d = nc.dram_tensor("labn", [1, NH], F32, kind="ExternalInput")
    labw_d = nc.dram_tensor("labw", [1, WWIN], F32, kind="ExternalInput")
    idlp_d = nc.dram_tensor("idlp", [P, WWIN], F32, kind="ExternalInput")
    out_d = nc.dram_tensor("out", [P, 4], F32, kind="ExternalOutput")

    with tile.TileContext(nc) as tc, ExitStack() as ctx:
        pool = ctx.enter_context(tc.tile_pool(name="sbuf", bufs=2))
        spool = ctx.enter_context(tc.tile_pool(name="scr", bufs=3))
        ppool = ctx.enter_context(tc.tile_pool(name="psum", bufs=1, space="PSUM"))
        dpool = ctx.enter_context(tc.tile_pool(name="dram", bufs=1, space="DRAM"))

        K = B // P  # contraction chunks

        def _body():
            # ---- loads ----
            # host pre-interleaves rows (chunk-major per partition) so each
            # tensor arrives in ONE DMA of 128 long contiguous lines; chunk k
            # of the contraction is then tile[:, k, :].
            lhsT_t = pool.tile([P, K, P], F16)
            rhsn_t = pool.tile([P, K, NH], F16)
            rhsw_t = pool.tile([P, K, WWIN], F16)
            # w-side chain is the longest: load rhsw first
            nc.sync.dma_start(rhsw_t[:], rhsw_d.ap())
            nc.sync.dma_start(lhsT_t[:], lhsT_d.ap())
            nc.sync.dma_start(rhsn_t[:], rhsn_d.ap())
            lhsT = [lhsT_t[:, k, :] for k in range(K)]
            rhsn = [rhsn_t[:, k, :] for k in range(K)]
            rhsw = [rhsw_t[:, k, :] for k in range(K)]
            laba = pool.tile([P, 1], F32)
            nc.sync.dma_start(laba[:], laba_d.ap())
            labn = pool.tile([1, NH], F32)
            nc.sync.dma_start(labn[:], labn_d.ap())
            labw = pool.tile([1, WWIN], F32)
            nc.sync.dma_start(labw[:], labw_d.ap())
            idlp = pool.tile([P, WWIN], F32)
            nc.sync.dma_start(idlp[:], idlp_d.ap())

            ones_r = pool.tile([1, P], F32)
            nc.vector.memset(ones_r[:], 1.0)
            ones_c = pool.tile([P, 1], F32)
            nc.vector.memset(ones_c[:], 1.0)
            zero_n = pool.tile([P, NH], F32)
            nc.vector.memset(zero_n[:], 0.0)
            zero_w = pool.tile([P, WWIN], F32)
            nc.vector.memset(zero_w[:], 0.0)

            # ---- column norms (squares + ones-matmul) ----
            def col_norms(rhs_chunks, width, tag):
                ps = ppool.tile([1, width], F32, tag=f"ps{tag}", name=f"sqps{tag}")
                for k in range(K):
                    sq = spool.tile([P, width], F32, tag=f"sq{tag}",
                                    name=f"sq{tag}_{k}")
                    nc.vector.tensor_tensor(
                        out=sq[:], in0=rhs_chunks[k], in1=rhs_chunks[k],
                        op=Alu.mult)
                    nc.tensor.matmul(ps[:], ones_c[:], sq[:],
                                     start=(k == 0), stop=(k == K - 1))
                row = pool.tile([1, width], F32, tag=f"sqrow{tag}",
                                name=f"sqrow{tag}")
                nc.vector.tensor_copy(row[:], ps[:])
                return row

            sqw_row = col_norms(rhsw, WWIN, "w")

            # ---- anchor norms: transpose sqw_row[self cols] via tiny DMA RT ----
            sqd = dpool.tile([1, WWIN], F32)
            nc.sync.dma_start(sqd[:], sqw_row[:])
            sq_src = sqd[:].copy()
            sq_src.ap = bass_rust.VecI64Pair([[1, P], [1, 1]])
            sq_src.offset = sq_src.offset + (W - 1)
            sq_a = pool.tile([P, 1], F32)
            nc.sync.dma_start(sq_a[:], sq_src)

            # ---- scaled lhsT (-2x) ----
            lhsTm2 = [pool.tile([P, P], F16, tag=f"lm2{k}", name=f"lm2{k}")
                      for k in range(K)]
            for k in range(K):
                nc.vector.tensor_scalar_mul(lhsTm2[k][:], lhsT[k], -2.0)

            # ---- gram + col-norm row accumulated on PE,
            # then d2 = max(psum + sq_a, 0), d = sqrt ----
            def dist(rhs_chunks, sq_row, width, tag):
                g = ppool.tile([P, width], F32, tag=f"g{tag}", name=f"g{tag}")
                for k in range(K):
                    nc.tensor.matmul(g[:], lhsTm2[k][:], rhs_chunks[k],
                                     start=(k == 0), stop=False)
                nc.tensor.matmul(g[:], ones_r[:], sq_row[:],
                                 start=False, stop=True)
                d2c = spool.tile([P, width], F32, tag=f"d2c{tag}",
                                 name=f"d2c{tag}")
                nc.vector.tensor_scalar(
                    out=d2c[:], in0=g[:], scalar1=sq_a[:], scalar2=0.0,
                    op0=Alu.add, op1=Alu.max)
                d = pool.tile([P, width], F32, tag=f"d{tag}", name=f"d{tag}")
                nc.scalar.activation(d[:], d2c[:], Act.Sqrt)
                return d

            d_w = dist(rhsw, sqw_row, WWIN, "w")

            # ---- label masks ----
            def lab_bcast(lab_row, width, tag):
                ps = ppool.tile([P, width], F32, tag=f"ps{tag}", name=f"lb{tag}")
                nc.tensor.matmul(ps[:], ones_r[:], lab_row[:],
                                 start=True, stop=True)
                return ps

            labn_b = lab_bcast(labn, NH, "n")
            eq_n = pool.tile([P, NH], F32)
            nc.vector.scalar_tensor_tensor(
                out=eq_n[:], in0=labn_b[:], scalar=laba[:], in1=zero_n[:],
                op0=Alu.is_equal, op1=Alu.add)

            labw_b = lab_bcast(labw, WWIN, "w")
            eq_w = pool.tile([P, WWIN], F32)
            csize = pool.tile([P, 1], F32)
            nc.vector.scalar_tensor_tensor(
                out=eq_w[:], in0=labw_b[:], scalar=laba[:], in1=zero_w[:],
                op0=Alu.is_equal, op1=Alu.add, accum_out=csize[:])

            # ---- positives window: DPw = d + (eq-1)*LARGE - 2*LARGE*self ----
            t_w = spool.tile([P, WWIN], F32, tag="tw")
            nc.vector.scalar_tensor_tensor(
                out=t_w[:], in0=eq_w[:], scalar=LARGE, in1=d_w[:],
                op0=Alu.mult, op1=Alu.add)
            dpw = pool.tile([P, WWIN], F32)
            nc.vector.tensor_tensor(out=dpw[:], in0=t_w[:], in1=idlp[:],
                                    op=Alu.subtract)

            # ---- band extraction via diagonal-stride DMA ----
            dpd = dpool.tile([P, WWIN], F32)
            nc.sync.dma_start(dpd[:], dpw[:])
            band_src = dpd[:].copy()
            band_src.ap = bass_rust.VecI64Pair([[WWIN + 1, P], [1, WB]])
            pos = pool.tile([P, WB], F32)
            nc.sync.dma_start(pos[:], band_src)
            pos_e = pool.tile([P, WB], F32)
            nc.vector.tensor_scalar_sub(pos_e[:], pos[:], EPS_TL)

            # ---- n-side (shorter chain): fills the band-RT wait gap ----
            sqn_row = col_norms(rhsn, NH, "n")
            d_n = dist(rhsn, sqn_row, NH, "n")
            ndn = pool.tile([P, NH], F32)
            nc.vector.scalar_tensor_tensor(
                out=ndn[:], in0=eq_n[:], scalar=-LARGE, in1=d_n[:],
                op0=Alu.mult, op1=Alu.subtract)

            # ---- hot loop ----
            sum_d = pool.tile([P, max(n_dve, 1)], F32)
            cnt_d = pool.tile([P, max(n_dve, 1)], F32)
            sum_a = pool.tile([P, max(n_act, 1)], F32)
            sgn_a = pool.tile([P, max(n_act, 1)], F32)
            if n_dve == 0:
                nc.vector.memset(sum_d[:], 0.0)
                nc.vector.memset(cnt_d[:], 0.0)

            jd = ja = 0
            for j in range(WB):
                use_act = (j * n_act) // WB != ((j + 1) * n_act) // WB
                if use_act:
                    scr1 = ppool.tile([P, NH], F32, tag="ascr",
                                      name=f"ascr1_{j}", bufs=2)
                    nc.scalar.activation(scr1[:], ndn[:], Act.Relu,
                                         bias=pos[:, j:j + 1], scale=1.0,
                                         accum_out=sum_a[:, ja:ja + 1])
                    scr2 = ppool.tile([P, NH], F32, tag="ascr",
                                      name=f"ascr2_{j}", bufs=2)
                    nc.scalar.activation(scr2[:], ndn[:], Act.Sign,
                                         bias=pos_e[:, j:j + 1], scale=1.0,
                                         accum_out=sgn_a[:, ja:ja + 1])
                    ja += 1
                else:
                    scr1 = spool.tile([P, NH], F32, tag="dscr",
                                      name=f"dscr1_{j}")
                    nc.vector.scalar_tensor_tensor(
                        out=scr1[:], in0=ndn[:], scalar=pos[:, j:j + 1],
                        in1=zero_n[:], op0=Alu.add, op1=Alu.max,
                        accum_out=sum_d[:, jd:jd + 1])
                    scr2 = spool.tile([P, NH], F32, tag="dscr",
                                      name=f"dscr2_{j}")
                    nc.vector.scalar_tensor_tensor(
                        out=scr2[:], in0=ndn[:], scalar=pos_e[:, j:j + 1],
                        in1=zero_n[:], op0=Alu.add, op1=Alu.is_gt,
                        accum_out=cnt_d[:, jd:jd + 1])
                    jd += 1
            assert ja == n_act and jd == n_dve

            # ---- final reductions ----
            out_t = pool.tile([P, 4], F32)
            r_sum_d = pool.tile([P, 1], F32)
            nc.vector.tensor_reduce(out=r_sum_d[:], in_=sum_d[:], axis=AX.X,
                                    op=Alu.add)
            r_sum_a = pool.tile([P, 1], F32)
            nc.vector.tensor_reduce(out=r_sum_a[:], in_=sum_a[:], axis=AX.X,
                                    op=Alu.add)
            nc.vector.tensor_tensor(out=out_t[:, 0:1], in0=r_sum_d[:],
                                    in1=r_sum_a[:], op=Alu.add)

            r_cnt_d = pool.tile([P, 1], F32)
            nc.vector.tensor_reduce(out=r_cnt_d[:], in_=cnt_d[:], axis=AX.X,
                                    op=Alu.add)
            r_sgn = pool.tile([P, 1], F32)
            nc.vector.tensor_reduce(out=r_sgn[:], in_=sgn_a[:], axis=AX.X,
                                    op=Alu.add)
            r_cnt_a = pool.tile([P, 1], F32)
            nc.vector.tensor_scalar(
                out=r_cnt_a[:], in0=r_sgn[:], scalar1=0.5,
                scalar2=float(NH // 2 * n_act), op0=Alu.mult, op1=Alu.add)
            nc.vector.tensor_tensor(out=out_t[:, 1:2], in0=r_cnt_d[:],
                                    in1=r_cnt_a[:], op=Alu.add)

            pc = pool.tile([P, 1], F32)
            nc.vector.tensor_scalar_sub(pc[:], csize[:], 1.0)
            nn_ = pool.tile([P, 1], F32)
            nc.vector.tensor_scalar(
                out=nn_[:], in0=csize[:], scalar1=-1.0, scalar2=float(B),
                op0=Alu.mult, op1=Alu.add)
            nc.vector.tensor_tensor(out=out_t[:, 2:3], in0=pc[:], in1=nn_[:],
                                    op=Alu.mult)
            nc.vector.tensor_copy(out_t[:, 3:4], csize[:])

            nc.sync.dma_start(out_d.ap(), out_t[:])

        if loop_iters is None:
            _body()
        else:
            with tc.For_i(0, loop_iters, 1):
                _body()

    nc.compile()
    return nc


def _ilv(a):
    """[512, x] -> [128, 4*x]: partition p holds rows p, p+128, p+256, p+384."""
    x = a.shape[1]
    return np.ascontiguousarray(
        a.reshape(4, P, x).transpose(1, 0, 2).reshape(P, 4 * x))


def _prepare(embeddings: np.ndarray, labels: np.ndarray):
    emb = np.ascontiguousarray(np.asarray(embeddings, dtype=np.float32))
    lab = np.asarray(labels)

    perm = np.argsort(lab, kind="stable")
    e_p = emb[perm]
    lab_p = lab[perm].astype(np.float32)

    _, counts = np.unique(lab_p, return_counts=True)
    W = int(counts.max())
    WWIN = P + 2 * (W - 1)

    e_pT = np.ascontiguousarray(e_p.T.astype(np.float16))   # [512 (d), 512 (x)]
    pad = W - 1
    e_padT = np.zeros((B, B + 2 * pad), dtype=np.float16)
    e_padT[:, pad:pad + B] = e_pT
    lab_pad = np.full((B + 2 * pad,), -1.0, dtype=np.float32)
    lab_pad[pad:pad + B] = lab_p

    # combined mask constant: LARGE everywhere + extra 2*LARGE on the
    # window-local self column (w == a + W - 1); same for every core.
    idlp = np.full((P, WWIN), LARGE, dtype=np.float32)
    for a in range(P):
        idlp[a, a + W - 1] += 2.0 * LARGE

    in_maps = []
    for c in range(N_CORES):
        b, h = c >> 1, c & 1
        bs = b * P
        in_maps.append({
            "lhsT": _ilv(e_pT[:, bs:bs + P]),
            "rhsn": _ilv(e_pT[:, h * NH:(h + 1) * NH]),
            "rhsw": _ilv(e_padT[:, bs:bs + WWIN]),
            "laba": np.ascontiguousarray(lab_p[bs:bs + P].reshape(P, 1)),
            "labn": np.ascontiguousarray(
                lab_p[h * NH:(h + 1) * NH].reshape(1, NH)),
            "labw": np.ascontiguousarray(lab_pad[bs:bs + WWIN].reshape(1, WWIN)),
            "idlp": idlp,
        })
    return W, in_maps


def _combine(outs):
    """outs: list of 8 [128, 4] arrays -> (loss, fraction_positive)."""
    loss_sum = 0.0
    num_pos = 0.0
    num_valid = 0.0
    for c in range(N_CORES):
        o = np.asarray(outs[c], dtype=np.float64)
        loss_sum += o[:, 0].sum()
        num_pos += o[:, 1].sum()
        if (c & 1) == 0:
            num_valid += o[:, 2].sum()
    loss = np.float32(loss_sum / (num_pos + 1e-5))
    frac = np.float32(num_pos / (num_valid + 1e-5))
    return (loss, frac)


def kernel(embeddings: np.ndarray, labels: np.ndarray):
    W, in_maps = _prepare(embeddings, labels)
    if W not in _cache:
        _cache[W] = _build(W)
    nc = _cache[W]
    res = run_bass_kernel_spmd(nc, in_maps, core_ids=list(range(N_CORES)))
    return _combine([res.results[c]["out"] for c in range(N_CORES)])



# revision 2
# speedup vs baseline: 1.0843x; 1.0843x over previous
"""BatchAllTripletLoss on 8 Trainium2 NeuronCores — padded class-slot grid.

Host layout:
  - Sort batch by label; pad each class to a slot of S = max_class_size
    columns (zero embeddings in pads). 32 classes x S=24 -> 768 padded
    anchor rows = 8 blocks x R=96 rows, one block per core; each core
    scores its 96 anchors against ALL 512 real negative columns.
  - Positives of anchor row r (slot k = r//S) are window cols [S*k, S*k+S)
    of the core's own padded cols -> band extraction is nslot FIXED
    rectangle ops (compile-time APs, SPMD-uniform, no DMA round trip).
  - The contraction gets an extra 34-row chunk: rows 0-31 carry
    1000*onehot(class) on both sides (gram accumulates 1e6 for same-class
    pairs = fused label mask, so sqrt input is never negative), row 32 is
    ones against |b|^2 (fused column-norm row), row 33 holds |a|^2 for the
    w-side row. num_valid is computed on host from label counts alone.

Device:
  - PE: grams (-2*A^T B + |b|^2 + 1e6*eq); anchor-norm bias via ACT/DVE.
  - n-side: dn = Sqrt(g + |a|^2) on ACT (f32), ndn = 32 - dn -> f16 (DVE
    tensor_scalar at 2x; the 32-shift keeps f16 error ~2e-3).
  - w-side: band2 = relu(g_w + |a|^2) per rectangle (DVE, from PSUM),
    dband = Sqrt (ACT), pos = dband + mb (f32; mb = -32 valid / -3032).
  - Hot loop over S slots x 2 single-src tensor_scalar ops (relu-sum
    accum + count accum) split DVE (fp16 4x) / ACT (Relu+Sign) / Pool.
  - No device reduction: raw accum columns are DMA'd out; host reduces.
"""
import sys
sys.path.insert(0, "/opt/trn_rl_repo")

import numpy as np
from contextlib import ExitStack

import concourse.bass as bass
import concourse.tile as tile
from concourse import bacc, mybir
from concourse.bass_utils import run_bass_kernel_spmd

F32 = mybir.dt.float32
F16 = mybir.dt.float16
Alu = mybir.AluOpType
Act = mybir.ActivationFunctionType
AX = mybir.AxisListType

B = 512
K = 4            # contraction chunks of 128 (512 dims)
KE = 34          # extra chunk rows: 32 onehot + ones + |a|^2
NB = 512         # negative columns per core
NH = 256         # half of NB
N_CORES = 8
SHIFT = 32.0
OH = 1000.0      # onehot amplitude -> 1e6 mask in the gram

_cache = {}


def _build(S: int, R: int, C: int, n_act: int = 4, n_pool: int = 0,
           loop_iters: int | None = None):
    """S = band width (max class size), R = rows per core (32-aligned
    class slots), C = number of classes."""
    SLOT = 32 * ((S + 31) // 32)
    nslot = R // SLOT
    n_dve = S - n_act - n_pool
    assert C <= 32 and R % SLOT == 0

    nc = bacc.Bacc("TRN2", target_bir_lowering=False, debug=False,
                   num_devices=N_CORES)

    lhsT_d = nc.dram_tensor("lhsT", [128, K * R], F16,
                            kind="ExternalInput")
    rhsa_d = nc.dram_tensor("rhsa", [128, K * NH], F16,
                            kind="ExternalInput")
    rhsb_d = nc.dram_tensor("rhsb", [128, K * NH], F16,
                            kind="ExternalInput")
    rhsx_d = nc.dram_tensor("rhsx", [33, NB + 2 * R], F16,
                            kind="ExternalInput")
    mb_d = nc.dram_tensor("mb", [R, S], F16, kind="ExternalInput")
    out_d = nc.dram_tensor("out", [R, 2 * S + 1], F32,
                           kind="ExternalOutput")

    with tile.TileContext(nc) as tc, ExitStack() as ctx:
        pool = ctx.enter_context(tc.tile_pool(name="sbuf", bufs=2))
        spool = ctx.enter_context(tc.tile_pool(name="scr", bufs=3))
        ppool = ctx.enter_context(tc.tile_pool(name="psum", bufs=1, space="PSUM"))

        def _body():
            # ---- input DMAs ----
            lhsT_t = pool.tile([128, K * R], F16)
            rhsa_t = pool.tile([128, K, NH], F16)
            rhsb_t = pool.tile([128, K, NH], F16)
            rhsx_t = pool.tile([33, NB + 2 * R], F16)
            mb_t = pool.tile([R, S], F16)
            nc.sync.dma_start(lhsT_t[:], lhsT_d.ap())
            nc.sync.dma_start(rhsa_t[:], rhsa_d.ap())
            nc.scalar.dma_start(rhsb_t[:], rhsb_d.ap())
            lhsT = [lhsT_t[:, k * R:(k + 1) * R] for k in range(K)]
            rhs = {0: [rhsa_t[:, k, :] for k in range(K)],
                   1: [rhsb_t[:, k, :] for k in range(K)]}
            lhs5 = rhsx_t[0:33, NB:NB + R]
            lhs5_ones = rhsx_t[32:33, NB:NB + R]
            lhs6_nrm = rhsx_t[32:33, NB + R:NB + 2 * R]
            rhs5 = {0: rhsx_t[0:33, 0:NH], 1: rhsx_t[0:33, NH:NB]}

            # ---- PE warmup: ramp the tensor engine out of low p-state ----
            nc.scalar.dma_start(rhsx_t[:], rhsx_d.ap())
            wsrc = pool.tile([128, NB], F16)
            nc.gpsimd.memset(wsrc[:], 0.0)
            nc.sync.dma_start(mb_t[:], mb_d.ap())
            dumt = pool.tile([1, 2], F32)
            nc.vector.memset(dumt[:], 1.0)
            one16 = pool.tile([33, 1], F16)
            nc.vector.memset(one16[:], 1.0)
            # pin the ACT table set (sqrt_and_others has all our funcs)
            dums = pool.tile([1, 2], F32)
            nc.scalar.activation(dums[:], dumt[:], Act.Sqrt)
            warm = ppool.tile([1, NB], F32, tag="warm", name="warm")
            for w in range(7):
                nc.tensor.matmul(warm[:], wsrc[:, 0:1], wsrc[:],
                                 start=True, stop=True)

            # ---- anchor-norm column: |a|^2 row as weights x ones column ----
            nrma = ppool.tile([R, 1], F32, tag="nrma", name="nrma")
            nc.tensor.matmul(nrma[:], lhs6_nrm, one16[32:33, :],
                             start=True, stop=True)
            nrma_s = pool.tile([R, 1], F32)
            nc.vector.tensor_copy(nrma_s[:], nrma[:])

            # ---- lhs scaled (-2x) ----
            lm2 = [pool.tile([128, R], F16, tag=f"lm2{k}", name=f"lm2{k}")
                   for k in range(K)]
            for k in range(K):
                nc.vector.tensor_scalar_mul(lm2[k][:], lhsT[k], -2.0)

            # ---- w-side gram + fused-relu band rectangles ----
            g_w = ppool.tile([R, R], F32, tag="gw", name="gw")
            for k in range(K):
                nc.tensor.matmul(g_w[:], lm2[k][:], lhsT[k],
                                 start=(k == 0), stop=False)
            nc.tensor.matmul(g_w[:], lhs5_ones, lhs6_nrm,
                             start=False, stop=True)
            band2 = pool.tile([R, S], F32)
            for k in range(nslot):
                nc.vector.tensor_scalar(
                    out=band2[k * SLOT:(k + 1) * SLOT, :],
                    in0=g_w[k * SLOT:(k + 1) * SLOT,
                            k * SLOT:k * SLOT + S],
                    scalar1=nrma_s[k * SLOT:(k + 1) * SLOT, :], scalar2=0.0,
                    op0=Alu.add, op1=Alu.max)
            dband = pool.tile([R, S], F32)
            nc.scalar.activation(dband[:], band2[:], Act.Sqrt)
            pos = pool.tile([R, S], F32)
            nc.vector.tensor_tensor(out=pos[:], in0=dband[:], in1=mb_t[:],
                                    op=Alu.add)
            negpos = pool.tile([R, S], F32)
            nc.vector.tensor_scalar_mul(negpos[:], pos[:], -1.0)

            # ---- n-side grams (onehot mask + col norms fused) ----
            ndn = pool.tile([R, NB], F16)
            for h in (0, 1):
                g = ppool.tile([R, NH], F32, tag=f"gn{h}", name=f"gn{h}")
                for k in range(K):
                    nc.tensor.matmul(g[:], lm2[k][:], rhs[h][k],
                                     start=(k == 0), stop=False)
                nc.tensor.matmul(g[:], lhs5, rhs5[h],
                                 start=False, stop=True)
                dn = spool.tile([R, NH], F32, tag=f"dn{h}", name=f"dn{h}")
                nc.scalar.activation(dn[:], g[:], Act.Sqrt,
                                     bias=nrma_s[:], scale=1.0)
                nc.vector.tensor_scalar(
                    out=ndn[:, h * NH:(h + 1) * NH], in0=dn[:],
                    scalar1=-1.0, scalar2=SHIFT,
                    op0=Alu.mult, op1=Alu.add)

            # ---- hot loop: S slots x (relu-sum + count) ----
            # DVE slots j < n_dve use the max-trick:
            #   sum(max(ndn, -p)) = sum(relu(ndn+p)) - NB*p   (host-corrected)
            # because tensor_scalar's accum reduces with op1 (must be add).
            # ACT slots (the last n_act) use native Relu/Sign accumulation.
            out_t = pool.tile([R, 2 * S + 1], F32)
            acc_a = pool.tile([R, 2 * max(n_act, 1)], F32)

            nc.vector.tensor_reduce(out=out_t[:, 2 * S:2 * S + 1],
                                    in_=pos[:, 0:n_dve], axis=AX.X,
                                    op=Alu.add)
            jd = ja = 0
            for j in range(S):
                pj = pos[:, j:j + 1]
                npj = negpos[:, j:j + 1]
                if j >= n_dve:
                    s1 = ppool.tile([R, NB], F32, tag="asc",
                                    name=f"asc1_{j}", bufs=2)
                    nc.scalar.activation(s1[:], ndn[:], Act.Relu,
                                         bias=pj, scale=1.0,
                                         accum_out=acc_a[:, 2 * ja:2 * ja + 1])
                    s2 = ppool.tile([R, NB], F32, tag="asc",
                                    name=f"asc2_{j}", bufs=2)
                    nc.scalar.activation(s2[:], ndn[:], Act.Sign,
                                         bias=pj, scale=1.0,
                                         accum_out=acc_a[:, 2 * ja + 1:2 * ja + 2])
                    ja += 1
                else:
                    s1 = spool.tile([R, NB], F16, tag="dsc", name=f"dsc1_{j}")
                    nc.vector.tensor_scalar(
                        out=s1[:], in0=ndn[:], scalar1=npj, scalar2=0.0,
                        op0=Alu.max, op1=Alu.add,
                        accum_out=out_t[:, 2 * jd:2 * jd + 1])
                    s2 = spool.tile([R, NB], F16, tag="dsc", name=f"dsc2_{j}")
                    nc.vector.tensor_scalar(
                        out=s2[:], in0=ndn[:], scalar1=npj, scalar2=0.0,
                        op0=Alu.is_gt, op1=Alu.add,
                        accum_out=out_t[:, 2 * jd + 1:2 * jd + 2])
                    jd += 1
            assert ja == n_act and jd == n_dve

            if n_act:
                nc.vector.tensor_copy(
                    out_t[:, 2 * n_dve:2 * (n_dve + n_act)], acc_a[:])

            nc.sync.dma_start(out_d.ap(), out_t[:])

        if loop_iters is None:
            _body()
        else:
            with tc.For_i(0, loop_iters, 1):
                _body()

    nc.compile()
    return nc


def _ilv(a, nchunk):
    """[nchunk*128 (contraction), x] -> [128, nchunk*x] chunk-interleaved."""
    x = a.shape[1]
    return np.ascontiguousarray(
        a.reshape(nchunk, 128, x).transpose(1, 0, 2).reshape(128, nchunk * x))


def _prepare(embeddings: np.ndarray, labels: np.ndarray):
    emb = np.ascontiguousarray(np.asarray(embeddings, dtype=np.float32))
    lab = np.asarray(labels)

    perm = np.argsort(lab, kind="stable")
    e_s = emb[perm]
    lab_s = lab[perm]
    classes, starts, counts = np.unique(lab_s, return_index=True,
                                        return_counts=True)
    C = len(classes)
    S = int(counts.max())
    SLOT = 32 * ((S + 31) // 32)
    spb = -(-C // N_CORES)            # class slots per block
    R = spb * SLOT
    assert R <= 128, f"padded rows per core {R} > 128"

    cls_of_col = np.searchsorted(starts, np.arange(B), side="right") - 1

    # padded anchors: class c -> slot c, rows [SLOT*c, SLOT*c+m_c)
    nP = N_CORES * R
    eP = np.zeros((nP, B), dtype=np.float32)
    cls_of_row = np.repeat(np.arange(-(-nP // SLOT)), SLOT)[:nP]
    live_row = np.zeros((nP,), dtype=bool)
    for c in range(C):
        eP[SLOT * c:SLOT * c + counts[c]] = \
            e_s[starts[c]:starts[c] + counts[c]]
        live_row[SLOT * c:SLOT * c + counts[c]] = True
    ePT = np.ascontiguousarray(eP.T).astype(np.float16)
    e_sT = np.ascontiguousarray(e_s.T).astype(np.float16)  # [512, 512]

    sqa_all = (ePT.astype(np.float32) ** 2).sum(0)         # [768]
    sqn = (e_sT.astype(np.float32) ** 2).sum(0)            # [512]

    # rhs extra chunks: onehot rows + |b|^2 in row 32
    def rhs_chunk(cols):
        ch = np.zeros((128, len(cols)), dtype=np.float16)
        ch[cls_of_col[cols], np.arange(len(cols))] = OH
        ch[32, :] = sqn[cols].astype(np.float16)
        return ch

    rhsa = _ilv(e_sT[:, :NH], K)
    rhsb = _ilv(e_sT[:, NH:], K)
    rhs_extra = np.ascontiguousarray(rhs_chunk(np.arange(NB))[0:33])

    num_valid = float((counts * (counts - 1) * (B - counts)).sum())

    in_maps = []
    for b in range(N_CORES):
        cols = np.arange(R * b, R * b + R)
        livec = live_row[cols]
        row_cls = np.minimum(cls_of_row[cols], C - 1)
        row_m = counts[row_cls]
        # lhs extra chunk: onehot rows (anchor class), ones row, |a|^2 row
        ch = np.zeros((33, R), dtype=np.float16)
        ch[row_cls, np.arange(R)] = np.where(livec, OH, 0.0)
        ch[32, :] = np.float16(1.0)
        ch6 = np.zeros((33, R), dtype=np.float16)
        ch6[32, :] = sqa_all[cols].astype(np.float16)
        lhs_chunks = ePT[:, cols].reshape(K, 128, R)
        lhsT = np.ascontiguousarray(np.concatenate(
            [lhs_chunks[k] for k in range(K)], axis=1))
        rhsx = np.ascontiguousarray(
            np.concatenate([rhs_extra, ch, ch6], axis=1))

        ii = np.tile(np.arange(SLOT), spb)
        jj = np.arange(S)[None, :]
        valid = ((jj < row_m[:, None]) & (jj != ii[:, None])
                 & (ii[:, None] < row_m[:, None]))
        mb = np.where(valid, -SHIFT, -3032.0).astype(np.float16)
        in_maps.append({
            "lhsT": lhsT,
            "rhsa": rhsa,
            "rhsb": rhsb,
            "rhsx": rhsx,
            "mb": mb,
        })
    return S, R, C, in_maps, num_valid


def _combine(outs, num_valid, S, n_act=4, n_pool=0):
    n_dve = S - n_act - n_pool
    loss_sum = 0.0
    num_pos = 0.0
    R = outs[0].shape[0]
    for c in range(N_CORES):
        o = np.asarray(outs[c], dtype=np.float64)
        # DVE max-trick columns need the +NB*sum(pos) correction
        sums = (o[:, 0:2 * n_dve:2].sum() + NB * o[:, 2 * S].sum()
                + o[:, 2 * n_dve:2 * (n_dve + n_act):2].sum())
        cnts = o[:, 1:2 * n_dve:2].sum()
        sgn = o[:, 2 * n_dve + 1:2 * (n_dve + n_act):2].sum()
        cnts += 0.5 * sgn + 0.5 * NB * n_act * R
        loss_sum += sums
        num_pos += cnts
    loss = np.float32(loss_sum / (num_pos + 1e-5))
    frac = np.float32(num_pos / (num_valid + 1e-5))
    return (loss, frac)


def kernel(embeddings: np.ndarray, labels: np.ndarray):
    S, R, C, in_maps, num_valid = _prepare(embeddings, labels)
    key = (S, R, C)
    if key not in _cache:
        _cache[key] = _build(S, R, C)
    nc = _cache[key]
    res = run_bass_kernel_spmd(nc, in_maps, core_ids=list(range(N_CORES)))
    return _combine([res.results[c]["out"] for c in range(N_CORES)],
                    num_valid, S)
